# revision 31
# baseline (speedup 1.0000x reference)
"""Trainium2 Bass kernel for 16-head attention (B=4, S=2048, D=1024).

Sharding: 8 cores = 4 batches x 2 head-groups. Core c handles batch c//2,
heads (c%2)*8 .. +8. Each core computes a partial projection output
[S, D] in bf16; the host sums the two head-group partials per batch and
adds b_proj. No collectives.

Main loop: 256 steps as 128 two-step groups. Per group the PE runs three
dense bursts, one per array-tiling mode, to exploit tile-level
concurrency: 4 score MMs (K=64; the row-tiled pair at tile (0,0)/(64,0)
runs both heads concurrently, ~233ns/pair), 4 attn@V MMs (M=64; the
col-tiled pair at (0,0)/(0,64) writes both heads into one PSUM bank's
partition halves, ~215ns/pair), then qkv/proj filler chains (full
128x128 mode). The exp of each step's [128,1024] score tile on the
scalar engine (~1.0us) sets the steady-state cadence; the A-stream of
attn@V runs LAG=16 steps behind the score stream on a deep pT pool.

Softmax denominators: V carries no ones-column (it would break the
64+64 col-tile pack). Instead the DVE folds the 16 exp'd pT tiles of
each block with an eager bf16 pair-add tree, and one ones-matmul per
head reduces the tree root across partitions (tree rounding averages
out across the 128-partition PE sum). The reduce-MM writes Z
*replicated* across the partition halves of the block's own attn bank
(evacuated right after), so one wide DVE reciprocal over the bank is
the whole broadcast; the normalize is a single [128,512] multiply.

Startup: the DMA rings boot at ~8-10us and pay per-transfer overhead,
so inputs are host-packed for single coalesced transfers: x^T arrives
as 4 chunk-major tensors (chunk = contiguous [128, 8k x 512] block of
the single xAll tile), wq/wk m=0 slices arrive partition-major. The
sync ring carries x chunks + wv, the (early-booting) gpsimd ring the
critical small weights + the last chunk + bulk weights; the ACT ring
stays near-empty since EXPs queue behind any DMA backlog on it. Dummy
matmuls on memset data bridge the PE from engine-boot to first-input
arrival so the HAM clock-gate never drops to 1.2 GHz. Filler chains
are emitted in arrival/deadline order; emission order also defines
dependency tracking, so every chain must be emitted before its first
consumer (a late chain races reads against uninitialized SBUF).
"""

import sys
import os

sys.path.insert(0, "/opt/trn_rl_repo")

import numpy as np
import ml_dtypes

BF = ml_dtypes.bfloat16

DIM = 1024
N_HEADS = 16
HD = 64
B = 4
S = 2048
HPC = 8          # heads per core
GC = HPC * HD    # 512 columns per head-group
N_CORES = 8
SCALE = HD ** -0.5
LAG = 18         # S-stream leads A-stream by this many steps (even)

_CACHE = {}


def _build_bass():
    import concourse.bass as bass
    import concourse.mybir as mybir
    import concourse.tile as tile
    from concourse import bacc

    f32 = mybir.dt.float32
    bf16 = mybir.dt.bfloat16
    EXP = mybir.ActivationFunctionType.Exp

    nc = bacc.Bacc("TRN2", target_bir_lowering=False, debug=False,
                   num_devices=N_CORES)

    # x^T arrives as 4 seq-chunk tensors so each [128, 512] k-tile slice
    # is one contiguous 128KB dram block (a chunked slice of a single
    # [DIM, S] tensor would be row-strided: 128 separate 1KB descriptors)
    # partition-major packed: xTc{c}[p, k*512+s] = x.T[k*128+p, c*512+s],
    # one coalesced 1MB DMA per chunk
    xTc = [nc.dram_tensor(f"xTc{c}", [128, 8 * 512], bf16,
                          kind="ExternalInput").ap() for c in range(4)]
    # wq/wk split into the m=0 column slice (gates the first chains) and
    # the rest, again for dram contiguity of k-tile slices
    # m=0 slices of wq/wk, partition-major [p, k*128+c]: one contiguous
    # DMA each (16 separate small transfers crawl on the SWDGE ring)
    wq0 = nc.dram_tensor("wq0", [128, DIM], bf16, kind="ExternalInput").ap()
    wk0 = nc.dram_tensor("wk0", [128, DIM], bf16, kind="ExternalInput").ap()
    wqr = nc.dram_tensor("wqr", [DIM, GC - 128], bf16,
                         kind="ExternalInput").ap()
    wkr = nc.dram_tensor("wkr", [DIM, GC - 128], bf16,
                         kind="ExternalInput").ap()
    wv = nc.dram_tensor("wv", [128, 8 * GC], bf16,
                        kind="ExternalInput").ap()
    wp = nc.dram_tensor("wp", [GC, DIM], bf16, kind="ExternalInput").ap()
    # per-partition qkv bias columns: bqk[p, m] = b_q[m*128+p] (m<4), b_k (m-4)
    bqk = nc.dram_tensor("bqk", [128, 8], f32, kind="ExternalInput").ap()
    # v bias broadcast across partitions (small)
    bvb = nc.dram_tensor("bvb", [128, GC], f32, kind="ExternalInput").ap()
    out = nc.dram_tensor("out", [S, DIM], bf16, kind="ExternalOutput").ap()

    KD = DIM // 128   # 8 k-tiles over D
    NQ = GC // 128    # 4 m-tiles over the 512 head-group columns
    ST = S // 128     # 16 seq tiles of 128
    NSTEP = 256

    with tile.TileContext(nc) as tc:
        with tc.tile_pool(name="const", bufs=1) as cp:
            ones_sb = cp.tile([128, 64], bf16, name="ones_sb")
            nc.any.memset(ones_sb[:], 1.0)
            warm_sb = cp.tile([128, 512], bf16, name="warm_sb")
            nc.any.memset(warm_sb[:], 0.001)
            # warm the ACT exp table during the input DMAs
            dummy = cp.tile([1, 16], bf16, name="dummy")
            nc.scalar.activation(dummy[:], ones_sb[0:1, 0:16], EXP)

            bqk_sb = cp.tile([128, 8], f32, name="bqk_sb")
            nc.scalar.dma_start(bqk_sb[:], bqk[:, :])

            # ---- input DMAs ----------------------------------------------
            # scalar ring: ONLY small early tiles (wk/wq m=0 slices gate the
            # first chains). Bulk traffic must stay off this ring: the ACT
            # engine drains it, and EXPs queue behind any DMA backlog here.
            # sync: xT by seq chunk (chunk 0 gates the first chains), then
            # wv. gpsimd: remaining weights.
            # chunk-major x layout: chunk c = contiguous [128, 4096] with
            # k-tile blocks of 512 inside; every consumer reads one
            # (c, k) block or a sub-range of it
            xAll = cp.tile([128, KD * S], bf16, name="xAll")

            def xs(k, c, lo=0, hi=512):
                return xAll[:, c * 4096 + k * 512 + lo:
                            c * 4096 + k * 512 + hi]
            wvAll = cp.tile([128, KD * GC], bf16, name="wvAll")
            wvs = [wvAll[:, k * GC:(k + 1) * GC] for k in range(KD)]
            wqs = [cp.tile([128, GC - 128], bf16, name=f"wqs{k}")
                   for k in range(KD)]
            wks = [cp.tile([128, GC - 128], bf16, name=f"wks{k}")
                   for k in range(KD)]

            # gpsimd ring: critical small weights first as single coalesced
            # transfers (the SWDGE ring pays ~0.5-1us per transfer), then
            # xTc3, then bulk weights. sync ring: xTc0, wv, xTc1, xTc2.
            # (the ACT ring is slow per-transfer; only bvb rides it)
            wk0_sb = cp.tile([128, DIM], bf16, name="wk0_sb")
            wq0_sb = cp.tile([128, DIM], bf16, name="wq0_sb")
            nc.gpsimd.dma_start(wk0_sb[:], wk0[:, :])
            nc.gpsimd.dma_start(wq0_sb[:], wq0[:, :])
            bvb_sb = cp.tile([128, GC], f32, name="bvb_sb")
            nc.scalar.dma_start(bvb_sb[:], bvb[:, :])
            # half-chunk transfers: consumers gate on a DMA's completion,
            # so halves restore progressive delivery for the first chains;
            # chunk-major dst keeps every transfer flat-contiguous
            for c in (0, 1, 2):
                for h in (0, 1):
                    nc.sync.dma_start(
                        xAll[:, c * 4096 + h * 2048:
                             c * 4096 + (h + 1) * 2048],
                        xTc[c][:, h * 2048:(h + 1) * 2048])
            for h in (0, 1):
                nc.gpsimd.dma_start(
                    xAll[:, 3 * 4096 + h * 2048:3 * 4096 + (h + 1) * 2048],
                    xTc[3][:, h * 2048:(h + 1) * 2048])
            for h in (0, 1):
                nc.sync.dma_start(wvAll[:, h * 2048:(h + 1) * 2048],
                                  wv[:, h * 2048:(h + 1) * 2048])
            for k in range(KD):
                nc.gpsimd.dma_start(wks[k][:], wkr[k * 128:(k + 1) * 128, :])
                nc.gpsimd.dma_start(wqs[k][:], wqr[k * 128:(k + 1) * 128, :])
            wps = []
            for k in range(NQ):
                t = cp.tile([128, DIM], bf16, name=f"wps{k}")
                nc.gpsimd.dma_start(t[:], wp[k * 128:(k + 1) * 128, :])
                wps.append(t)

            QT = [cp.tile([128, S], bf16, name=f"QT{m}") for m in range(NQ)]
            KT = [cp.tile([128, S], bf16, name=f"KT{m}") for m in range(NQ)]
            # V tiles: per head 64 plain cols (no ones column)
            Vt = [cp.tile([128, GC], bf16, name=f"Vt{s}") for s in range(ST)]
            OT = [cp.tile([128, S], bf16, name=f"OT{m}") for m in range(NQ)]

            with tc.tile_pool(name="ps", bufs=1, space="PSUM") as psp, \
                 tc.tile_pool(name="pbuf", bufs=LAG + 2) as pbufp, \
                 tc.tile_pool(name="tre", bufs=3) as trep, \
                 tc.tile_pool(name="un", bufs=2) as unp, \
                 tc.tile_pool(name="rr", bufs=2) as rrp, \
                 tc.tile_pool(name="stg", bufs=3) as stgp:

                def ps_s(name):          # scores: 2 banks x2
                    return psp.tile([128, 1024], f32, tag="s", bufs=2,
                                    name=name)

                def ps_f(name):          # qkv/proj half-chains: 1 bank x2
                    return psp.tile([128, 512], f32, tag="f", bufs=2,
                                    name=name)

                # ---- V units: per seq-tile, split into column halves so
                # the warmup only computes the hp 0/1 half (block 0 needs
                # V cols 0:128 only); the hp 2/3 half runs in mid-kernel
                # slack before a=128
                def v_units(s, half):
                    cs = slice(half * 256, (half + 1) * 256)
                    state = {}

                    def part(k0):
                        def go():
                            if k0 == 0:
                                state["ps"] = ps_f(f"v{s}h{half}")
                            ps = state["ps"]
                            for k in range(k0, k0 + 2):
                                nc.tensor.matmul(
                                    ps[:, 0:256],
                                    lhsT=xs(k, s // 4, (s % 4) * 128,
                                            (s % 4) * 128 + 128),
                                    rhs=wvs[k][:, cs],
                                    start=(k == 0), stop=(k == KD - 1))
                            if k0 == KD - 2:
                                nc.vector.tensor_add(Vt[s][:, cs],
                                                     ps[:, 0:256],
                                                     bvb_sb[:, cs])
                        return go

                    for k0 in range(0, KD, 2):
                        yield part(k0)

                # ---- Q/K half-chain units (4 x 2-matmul units) ------------
                def qk_units(is_k, m, n2, h):
                    w0 = wk0_sb if is_k else wq0_sb
                    ws = wks if is_k else wqs
                    dst = KT[m] if is_k else QT[m]
                    bcol = bqk_sb[:, 4 + m:5 + m] if is_k else \
                        bqk_sb[:, m:m + 1]
                    state = {}

                    def part(k0):
                        def go():
                            if k0 == 0:
                                state["ps"] = ps_f(
                                    f"{'k' if is_k else 'q'}{m}{n2}{h}")
                            ps = state["ps"]
                            for k in range(k0, k0 + 2):
                                lh = (w0[:, k * 128:(k + 1) * 128]
                                      if m == 0 else
                                      ws[k][:, (m - 1) * 128:m * 128])
                                nc.tensor.matmul(
                                    ps[:],
                                    lhsT=lh,
                                    rhs=xs(k, n2 * 2 + h),
                                    start=(k == 0), stop=(k == KD - 1))
                            if k0 == KD - 2:
                                nc.vector.tensor_scalar_add(
                                    dst[:, (n2 * 2 + h) * 512:
                                        (n2 * 2 + h + 1) * 512],
                                    ps[:], bcol)
                        return go

                    for k0 in range(0, KD, 2):
                        yield part(k0)

                def emit_qk_half(is_k, m, n2, h):
                    for u in qk_units(is_k, m, n2, h):
                        u()

                # ---- proj half units (2 x 2-matmul units) -----------------
                tail_mode = [False]
                tail_bank = [False]
                tail_alt = [0]

                def proj_units(mt, h):
                    state = {}

                    def part(k0):
                        def go():
                            if k0 == 0:
                                if tail_bank[0] and tail_alt[0] % 2:
                                    state["ps"] = psp.tile(
                                        [128, 512], f32, tag="o", bufs=2,
                                        name=f"pjo{mt}{h}")
                                else:
                                    state["ps"] = ps_f(f"pj{mt}{h}")
                                tail_alt[0] += 1
                            ps = state["ps"]
                            for k in range(k0, k0 + 2):
                                nc.tensor.matmul(
                                    ps[:],
                                    lhsT=OT[k][:, mt * 128:(mt + 1) * 128],
                                    rhs=wps[k][:, h * 512:(h + 1) * 512],
                                    start=(k == 0), stop=(k == NQ - 1))
                            if k0 == NQ - 2:
                                ob = stgp.tile([128, 512], bf16, tag="ob",
                                               name=f"ob{mt}{h}")
                                if tail_mode[0] and tail_alt[0] % 2:
                                    nc.scalar.copy(ob[:], ps[:])
                                else:
                                    nc.vector.tensor_copy(ob[:], ps[:])
                                nc.sync.dma_start(
                                    out[mt * 128:(mt + 1) * 128,
                                        h * 512:(h + 1) * 512], ob[:])
                        return go

                    for k0 in range(0, NQ, 2):
                        yield part(k0)

                # ---- PE warmup: dummy MMs on memset data keep the HAM
                # clock-gate open until the input DMAs land (~11.5us);
                # without this the whole warmup phase runs at 1.2 GHz
                wrm = ps_f("warm")
                for i in range(14):
                    nc.tensor.matmul(wrm[:], lhsT=warm_sb[:, 0:128],
                                     rhs=warm_sb[:], start=True, stop=True)
                nc.vector.tensor_copy(warm_sb[0:1, 0:16], wrm[0:1, 0:16])

                # ---- startup: just enough for the first scores ------------
                emit_qk_half(True, 0, 0, 0)    # KT[0] sk 0:512  (j=0..3)
                emit_qk_half(False, 0, 0, 0)   # QT[0] sq 0:512

                # ---- filler generator (ordered by first-need tick) --------
                def gen_fillers():
                    # emission deadlines: scores for g are emitted at group
                    # g//2; attnV for a at group (a+LAG)//2; a unit must be
                    # YIELDED before its consumer's emission group
                    yield from qk_units(True, 0, 0, 1)    # j=4..7 (xTc1)
                    yield from qk_units(False, 0, 0, 1)   # sq 512:1024 @g16
                    yield from qk_units(True, 0, 1, 0)    # j=8..11 (xTc2)
                    yield from qk_units(True, 0, 1, 1)    # j=12..15 (xTc3)
                    for s in range(0, 4):
                        yield from v_units(s, 0)
                    yield from qk_units(False, 0, 1, 0)   # sq 1024:1536 @g32
                    for s in range(4, 8):
                        yield from v_units(s, 0)
                    yield from qk_units(False, 0, 1, 1)   # sq 1536:2048 @g48
                    for s in range(8, ST):
                        yield from v_units(s, 0)
                    for m in (1,):
                        for n2, h, is_k in ((0, 0, True), (0, 1, True),
                                            (0, 0, False), (0, 1, False),
                                            (1, 0, True), (1, 1, True),
                                            (1, 0, False), (1, 1, False)):
                            yield from qk_units(is_k, m, n2, h)
                    # V hp2/3 halves: needed from a=128 (emitted group 73+);
                    # interleave with the m=2 chains (needed g=128+)
                    for s in range(ST):
                        if s % 4 == 0:
                            m = 2
                            n2, h = divmod(s // 4, 2)
                            yield from qk_units(True, m, n2, h)
                            yield from qk_units(False, m, n2, h)
                        yield from v_units(s, 1)
                    for m in (3,):
                        for n2, h, is_k in ((0, 0, True), (0, 1, True),
                                            (0, 0, False), (0, 1, False),
                                            (1, 0, True), (1, 1, True),
                                            (1, 0, False), (1, 1, False)):
                            yield from qk_units(is_k, m, n2, h)

                fillers = gen_fillers()
                proj_queue = []

                def pull(n_units):
                    for _ in range(n_units):
                        u = next(fillers, None)
                        if u is not None:
                            u()
                        elif proj_queue:
                            proj_queue.pop(0)()

                # ---- block-boundary (normalize) machinery -----------------
                # pending = (hp, n, o_tile, acc_tile, state-dict)
                # codes: 1 u-copy, 2 Z-MMs (col mode, after attnV burst),
                # 5 wide recip of the replicated Z, 7 OT-mul, 9 proj release
                def bdy_ops(pend, j):
                    hp, n, o, acc, st = pend
                    sq = slice(n * 512, (n + 1) * 512)
                    if j == 1:
                        u = unp.tile([128, 512], f32, tag="u",
                                     name=f"u{hp}{n}")
                        if tail_mode[0]:
                            nc.scalar.copy(u[:], o[:])
                        else:
                            nc.vector.tensor_copy(u[:], o[:])
                        st["u"] = u
                    elif j == 2:
                        # Z replicated into the evacuated o bank (col mode):
                        # partitions 0:64 all hold Z_A, 64:128 all hold Z_B
                        nc.tensor.matmul(o[0:64, :], lhsT=ones_sb[:, :],
                                         rhs=acc[:, 0:512],
                                         start=True, stop=True)
                        nc.tensor.matmul(o[64:128, :], lhsT=ones_sb[:, :],
                                         rhs=acc[:, 512:1024],
                                         start=True, stop=True)
                    elif j == 5:
                        # the replication makes the recip broadcast free:
                        # one wide DVE reciprocal over the whole bank
                        pbr = rrp.tile([128, 512], f32, tag="pbr",
                                       name=f"pbr{hp}{n}")
                        nc.vector.reciprocal(pbr[:], o[:])
                        st["pbr"] = pbr
                    elif j == 7:
                        nc.vector.tensor_mul(OT[hp][:, sq], st["u"][:],
                                             st["pbr"][:])
                    elif j == 9:
                        if hp == NQ - 1:
                            for mt in range(4 * n, 4 * n + 4):
                                for hh in range(2):
                                    proj_queue.extend(proj_units(mt, hh))
                        return True
                    return False

                # ---- main loop: 2-step groups -----------------------------
                pending = None
                pTs = {}
                l1, l2, l3 = {}, {}, {}
                ablk = {}

                for g2 in range(0, NSTEP + LAG, 2):
                    # S-stream: scores burst (row-tiled 64-mode)
                    for g in (g2, g2 + 1):
                        if g >= NSTEP:
                            continue
                        hp, n, j = g // 64, (g // 16) % 4, g % 16
                        sq = slice(n * 512, (n + 1) * 512)
                        sk = slice(j * 128, (j + 1) * 128)
                        sS = ps_s(f"sS{g}")
                        nc.tensor.matmul(
                            sS[:, 0:512], lhsT=KT[hp][0:64, sk],
                            rhs=QT[hp][0:64, sq],
                            start=True, stop=True)
                        nc.tensor.matmul(
                            sS[:, 512:1024], lhsT=KT[hp][64:128, sk],
                            rhs=QT[hp][64:128, sq],
                            start=True, stop=True)
                        pT = pbufp.tile([128, 1024], bf16, tag="p",
                                        name=f"pT{g}")
                        nc.scalar.activation(pT[:], sS[:], EXP, scale=SCALE)
                        pTs[g] = pT
                    # A-stream: attnV burst (col-tiled 64-mode)
                    zmm_due = None
                    for a in (g2 - LAG, g2 - LAG + 1):
                        if a < 0 or a >= NSTEP:
                            continue
                        hp, n, j = a // 64, (a // 16) % 4, a % 16
                        if j == 0:
                            ablk["o"] = psp.tile([128, 512], f32, tag="o",
                                                 bufs=2, name=f"o{a}")
                        o = ablk["o"]
                        pT = pTs[a]
                        ha = hp * 2
                        nc.tensor.matmul(
                            o[0:64, :],
                            lhsT=Vt[j][:, ha * 64:ha * 64 + 64],
                            rhs=pT[:, 0:512],
                            start=(j == 0), stop=(j == ST - 1))
                        nc.tensor.matmul(
                            o[64:128, :],
                            lhsT=Vt[j][:, ha * 64 + 64:ha * 64 + 128],
                            rhs=pT[:, 512:1024],
                            start=(j == 0), stop=(j == ST - 1))
                        if pending is not None and j == 2:
                            zmm_due = pending
                        # eager bf16 pair-add tree for softmax denominators
                        if j % 2 == 1:
                            t1 = trep.tile([128, 1024], bf16, tag="l1",
                                           name=f"l1{a}")
                            nc.vector.tensor_add(t1[:], pTs[a - 1][:],
                                                 pT[:])
                            l1[a // 2] = t1
                            pTs.pop(a - 1)
                            pTs.pop(a)
                        if j % 4 == 3:
                            t2 = trep.tile([128, 1024], bf16, tag="l2",
                                           name=f"l2{a}")
                            nc.vector.tensor_add(t2[:], l1.pop(a // 2 - 1)[:],
                                                 l1.pop(a // 2)[:])
                            l2[a // 4] = t2
                        if j % 8 == 7:
                            t3 = trep.tile([128, 1024], bf16, tag="l3",
                                           name=f"l3{a}")
                            nc.vector.tensor_add(t3[:], l2.pop(a // 4 - 1)[:],
                                                 l2.pop(a // 4)[:])
                            l3[a // 8] = t3
                        if j == ST - 1:
                            acc = trep.tile([128, 1024], bf16, tag="l4",
                                            name=f"l4{a}")
                            nc.vector.tensor_add(acc[:], l3.pop(a // 8 - 1)[:],
                                                 l3.pop(a // 8)[:])
                            ablk["done"] = (hp, n, o, acc, {})
                        # boundary ops for the previous block (not Z MMs,
                        # those are placed next to the attnV burst)
                        if pending is not None and j in (1, 5, 7, 9):
                            if bdy_ops(pending, j):
                                pending = None
                        if j == ST - 1:
                            pending = ablk["done"]
                    # Z reduce MMs ride in the col-tiled mode right after
                    # the attnV burst
                    if zmm_due is not None:
                        bdy_ops(zmm_due, 2)
                    # fillers (full 128x128 mode)
                    if g2 >= NSTEP:
                        tail_mode[0] = True
                        pull(3)     # drain: proj units when released
                    elif g2 < LAG:
                        pull(6)
                    elif g2 < 40:
                        pull(5)
                    elif g2 < 64:
                        pull(3)
                    elif g2 % 4 == 0:
                        pull(1)
                    else:
                        pull(2)

                # tail: last block's normalize + remaining proj
                tail_mode[0] = True
                for j in (1, 2, 5, 7, 9):
                    bdy_ops(pending, j)
                pending = None
                tail_bank[0] = True
                while proj_queue:
                    proj_queue.pop(0)()
                pull(1000)
    nc.compile()
    return nc


def _get_nc():
    if "nc" not in _CACHE:
        _CACHE["nc"] = _build_bass()
    return _CACHE["nc"]


def _pm(a):
    """[8*128, C] -> partition-major [128, 8*C] (tile k at cols k*C)."""
    a = np.ascontiguousarray(a)
    return np.ascontiguousarray(
        a.reshape(8, 128, a.shape[1]).transpose(1, 0, 2)
        .reshape(128, 8 * a.shape[1]))


def _in_maps(x, w_qkv, b_qkv, w_proj, b_proj):
    x = np.asarray(x, np.float32)
    w_qkv = np.asarray(w_qkv, np.float32)
    b_qkv = np.asarray(b_qkv, np.float32)
    w_proj = np.asarray(w_proj, np.float32)

    maps = []
    for c in range(N_CORES):
        b, g = divmod(c, 2)
        cols = slice(g * GC, (g + 1) * GC)
        wqs = w_qkv[:, 0 * DIM:1 * DIM][:, cols]
        wks = w_qkv[:, 1 * DIM:2 * DIM][:, cols]
        wvs = w_qkv[:, 2 * DIM:3 * DIM][:, cols]
        bqs = b_qkv[0 * DIM:1 * DIM][cols]
        bks = b_qkv[1 * DIM:2 * DIM][cols]
        bvs = b_qkv[2 * DIM:3 * DIM][cols]
        rows = slice(g * GC, (g + 1) * GC)
        bqk = np.concatenate([bqs.reshape(4, 128).T,
                              bks.reshape(4, 128).T], axis=1)
        xTb = np.ascontiguousarray(x[b].T).astype(BF)
        maps.append({
            "xTc0": _pm(xTb[:, 0:512]),
            "xTc1": _pm(xTb[:, 512:1024]),
            "xTc2": _pm(xTb[:, 1024:1536]),
            "xTc3": _pm(xTb[:, 1536:2048]),
            "wq0": np.ascontiguousarray(
                wqs[:, 0:128].astype(BF).reshape(8, 128, 128)
                .transpose(1, 0, 2).reshape(128, 1024)),
            "wk0": np.ascontiguousarray(
                wks[:, 0:128].astype(BF).reshape(8, 128, 128)
                .transpose(1, 0, 2).reshape(128, 1024)),
            "wqr": np.ascontiguousarray(wqs[:, 128:].astype(BF)),
            "wkr": np.ascontiguousarray(wks[:, 128:].astype(BF)),
            "wv": _pm(wvs.astype(BF)),
            "wp": w_proj[rows, :].astype(BF),
            "bqk": np.ascontiguousarray(bqk, dtype=np.float32),
            "bvb": np.broadcast_to(bvs, (128, GC)).copy(),
        })
    return maps


def kernel(x, w_qkv, b_qkv, w_proj, b_proj, _trace=False):
    import time
    from concourse import bass_utils
    nc = _get_nc()
    maps = _in_maps(x, w_qkv, b_qkv, w_proj, b_proj)
    try:
        res = bass_utils.run_bass_kernel_spmd(nc, maps,
                                              core_ids=list(range(N_CORES)),
                                              trace=_trace)
    except Exception:
        # a previously wedged device usually clears after one failed
        # attempt; retry once
        time.sleep(5)
        res = bass_utils.run_bass_kernel_spmd(nc, maps,
                                              core_ids=list(range(N_CORES)),
                                              trace=_trace)
    _CACHE["last_result"] = res
    b_proj = np.asarray(b_proj, np.float32)
    outs = np.empty((B, S, DIM), np.float32)
    for b in range(B):
        outs[b] = (res.results[2 * b]["out"].astype(np.float32)
                   + res.results[2 * b + 1]["out"].astype(np.float32)
                   + b_proj)
    return outs
